# revision 3
# baseline (speedup 1.0000x reference)
"""Trainium2 Bass kernel: 5-point Jacobi stencil with Dirichlet boundary.

out[b,0,i,j] = 0.25*(v[i-1,j]+v[i+1,j]+v[i,j-1]+v[i,j+1]) + cof*f[i,j]
(interior; boundary = 0), v = u with boundary forced to 0,
cof = -(1/1023)^2/4. Data-parallel over batch: 2 images per core, 8 cores.

Per-core layout: image [1024,1024] -> SBUF tile [128 partitions, 8 rows + 2
pad elems]; partition p holds rows 8p..8p+7, loaded as ONE SWDGE DMA that
casts fp32->bf16 in flight (halves SBUF-side DMA bytes; the DRAM side runs
at the 16-SDMA-engine line rate ~27GB/s/engine either way, which is the
kernel's binding constraint: 12MB/image of fp32 DRAM traffic).

The whole stencil runs on the otherwise-idle TensorEngine as accumulating
512-col identity matmuls into PSUM: left/right taps are +-1-element shifts
of the bf16 streaming operand (1 col/cycle), up/down taps are +-1024
shifts, the two cross-partition taps (up-tap of local row 0 / down-tap of
local row 7) use partition-shift stationaries S_up/S_dn against the
neighbor partition's edge-row columns, and the f term accumulates via a
(4*cof*I) stationary. PSUM accumulates in fp32, so the only error is bf16
input rounding (~2.3e-3 relative, well inside the 2e-2 gate).

DVE only drains PSUM (oc = 0.25 * psum, fp32), applies the v-column
boundary corrections, and zeroes the output boundary; stores are plain
fp32 HWDGE DMAs (separate ring from the SWDGE load queue). No engine ever
writes into the u tile (avoids an engine-write -> PE-read ordering hazard
observed with strided memsets): boundary rows are zeroed by DMA from a
zero line, the 1-elem pads hold garbage that only feeds output boundary
columns (memset after the drain), and the column Dirichlet condition is
applied as post-drain corrections:
    oc[:, r, 1]    -= 0.25 * u[:, r, 0]
    oc[:, r, 1022] -= 0.25 * u[:, r, 1023]
The last image splits its final 2 rows into 1-row chunks so the closing
drain/store pipeline instead of serializing.

Like v6 (banded u load, PE identity-matmul stencil) but no engine ever
writes into ut: boundary rows 0/1023 are zeroed by DMA from a zero line,
the 1-elem pads hold garbage (they only feed output boundary columns that
are memset after the drain), and the v-column-boundary condition is applied
as post-drain corrections on oc:
    oc[:, r, 1]    -= 0.25 * u[:, r, 0]
    oc[:, r, 1022] -= 0.25 * u[:, r, 1023]
"""
import numpy as np
import concourse.bacc as bacc
import concourse.bass as bass
import concourse.mybir as mybir
from concourse.ap import AP
from concourse.tile import TileContext
from concourse.bass_utils import run_bass_kernel_spmd

N_CORES = 8
B_FULL = 16
H = 1024
W = 1024
IMGS = B_FULL // N_CORES
P = 128
RPP = H // P
FREE = RPP * W + 2
COF = float(np.float32(-((1.0 / 1023.0) ** 2) / 4.0))
F32 = mybir.dt.float32
BF16 = mybir.dt.bfloat16

add = mybir.AluOpType.add
mult = mybir.AluOpType.mult

_cache = {}


def _build(repeat=1, CHUNKS=4, NSLOT=0, DRAIN="dve", OD=F32, PSBUFS=2,
           FBUFS=3, OBUFS=4, WIN=512, USPLIT=1, FSPLIT=1, OENG="sync", TAIL1=2):
    nc = bacc.Bacc("TRN2", target_bir_lowering=False)
    u_d = nc.dram_tensor("u", [IMGS, 1, H, W], F32, kind="ExternalInput")
    f_d = nc.dram_tensor("f", [IMGS, 1, H, W], F32, kind="ExternalInput")
    id_d = nc.dram_tensor("ident", [P, 3 * P], F32, kind="ExternalInput")
    nout = NSLOT if NSLOT else IMGS
    o_d = nc.dram_tensor("out", [nout, 1, H, W], F32, kind="ExternalOutput")

    n_imgs = IMGS * repeat
    slot = [0]
    o_eng = {"sync": nc.sync, "scalar": nc.scalar}[OENG] if OD == F32 else nc.gpsimd
    drain_eng = nc.vector if DRAIN == "dve" else nc.scalar

    with TileContext(nc) as tc:
        with (
            tc.tile_pool(name="upool", bufs=2) as upool,
            tc.tile_pool(name="fpool", bufs=FBUFS) as fpool,
            tc.tile_pool(name="opool", bufs=OBUFS) as opool,
            tc.tile_pool(name="zpool", bufs=1) as zpool,
            tc.tile_pool(name="pspool", bufs=PSBUFS, space="PSUM") as pspool,
        ):
            state = {}

            def issue_uload(ib):
                b = ib % IMGS
                u4 = u_d[b, 0, :, :]
                ut = upool.tile([P, FREE], BF16, name=f"ut{ib}", tag="ut")
                u_r = u4.rearrange("(p r) j -> p (r j)", r=RPP)
                nsp = max(1, USPLIT)
                step = RPP * W // nsp
                for si in range(nsp):
                    nc.gpsimd.dma_start(
                        out=ut[:, 1 + si * step : 1 + (si + 1) * step],
                        in_=u_r[:, si * step : (si + 1) * step],
                    )
                return ut

            def issue_setup():
                ztc = zpool.tile([1, W], BF16, name="ztc")
                nc.vector.memset(ztc, 0.0)
                zto = zpool.tile([1, W], OD, name="zto")
                nc.vector.memset(zto, 0.0)
                id_f = zpool.tile([P, 3 * P], F32, name="id_f")
                nc.sync.dma_start(out=id_f, in_=id_d[:, :])
                id_b = zpool.tile([P, P], BF16, name="id_b")
                nc.vector.tensor_scalar(out=id_b, in0=id_f[:, 0:P], scalar1=1.0,
                                        scalar2=None, op0=mult)
                cid_b = zpool.tile([P, P], BF16, name="cid_b")
                nc.vector.tensor_scalar(out=cid_b, in0=id_f[:, 0:P],
                                        scalar1=4.0 * COF, scalar2=None, op0=mult)
                sup_b = zpool.tile([P, P], BF16, name="sup_b")
                nc.vector.tensor_scalar(out=sup_b, in0=id_f[:, P : 2 * P],
                                        scalar1=1.0, scalar2=None, op0=mult)
                sdn_b = zpool.tile([P, P], BF16, name="sdn_b")
                nc.vector.tensor_scalar(out=sdn_b, in0=id_f[:, 2 * P : 3 * P],
                                        scalar1=1.0, scalar2=None, op0=mult)
                state.update(ztc=ztc, zto=zto, id_b=id_b, cid_b=cid_b,
                             sup_b=sup_b, sdn_b=sdn_b)

            def issue_vrows(ut):
                # v rows 0 / 1023 zeroed via DMA (no engine writes into ut)
                nc.sync.dma_start(out=ut[0:1, 1 : W + 1], in_=state["ztc"])
                nc.sync.dma_start(
                    out=ut[127:128, 1 + 7 * W : 1 + 8 * W], in_=state["ztc"]
                )

            def issue_chunks(ib, ut, final=False):
                b = ib % IMGS
                f_img = f_d[b, 0, :, :].rearrange("(p r) j -> p (r j)", r=RPP)
                if NSLOT:
                    ob = slot[0]
                    slot[0] = (slot[0] + 1) % NSLOT
                else:
                    ob = b
                o_img = o_d[ob, 0, :, :].rearrange("(p r) j -> p (r j)", r=RPP)
                id_b, cid_b = state["id_b"], state["cid_b"]
                sup_b, sdn_b = state["sup_b"], state["sdn_b"]

                ft = fpool.tile([P, RPP * W], BF16, name=f"ft{ib}", tag="ft")
                fstep = RPP * W // max(1, FSPLIT)
                for sfi in range(max(1, FSPLIT)):
                    nc.gpsimd.dma_start(
                        out=ft[:, sfi * fstep : (sfi + 1) * fstep],
                        in_=f_img[:, sfi * fstep : (sfi + 1) * fstep],
                    )

                nr0 = RPP // CHUNKS
                chunk_list = [(ci * nr0, nr0) for ci in range(CHUNKS)]
                if final:
                    keep, tail = [], []
                    for r0c, nrc in chunk_list:
                        if r0c + nrc > RPP - TAIL1:
                            tail += [(r0c + k, 1) for k in range(nrc)]
                        else:
                            keep.append((r0c, nrc))
                    chunk_list = keep + tail
                for ci, (r0, nr) in enumerate(chunk_list):
                    cw = nr * W
                    base = 1 + r0 * W
                    fc = ft[:, r0 * W : r0 * W + cw]
                    pt = pspool.tile([P, cw], F32, name=f"pt{ib}_{ci}", tag="pt")
                    oc = opool.tile([P, cw], OD, name=f"oc{ib}_{ci}", tag="oc")

                    up_lo = W if r0 == 0 else 0
                    dn_hi = cw - W if r0 + nr == RPP else cw
                    for w in range(cw // WIN):
                        s0, s1 = w * WIN, (w + 1) * WIN

                        def mm(dst0, dst1, stat, src_tile, src0, start=False,
                               stop=False):
                            nc.tensor.matmul(
                                pt[:, dst0:dst1], stat,
                                src_tile[:, src0 : src0 + (dst1 - dst0)],
                                start=start, stop=stop,
                                skip_group_check=True,
                            )
                        mm(s0, s1, id_b, ut, base + s0 - 1, start=True)
                        mm(s0, s1, id_b, ut, base + s0 + 1)
                        a, bnd = max(s0, up_lo), s1
                        if a < bnd:
                            mm(a, bnd, id_b, ut, base + a - W)
                        if r0 == 0 and s0 < W:
                            a, bnd = s0, min(s1, W)
                            mm(a, bnd, sup_b, ut, 1 + 7 * W + a)
                        a, bnd = s0, min(s1, dn_hi)
                        if a < bnd:
                            mm(a, bnd, id_b, ut, base + a + W)
                        if r0 + nr == RPP and s1 > cw - W:
                            a, bnd = max(s0, cw - W), s1
                            mm(a, bnd, sdn_b, ut, 1 + (a - (cw - W)))
                        mm(s0, s1, cid_b, fc, s0, stop=True)

                    if DRAIN == "dve":
                        nc.vector.tensor_scalar(out=oc, in0=pt, scalar1=0.25,
                                                scalar2=None, op0=mult)
                    else:
                        nc.scalar.mul(oc, pt, 0.25)

                    # v-column boundary corrections (l-tap of col 1 read u
                    # col 0; r-tap of col 1022 read u col 1023 — subtract)
                    ocv = oc.rearrange("p (r j) -> p r j", j=W)
                    utc = ut[:, base : base + cw].rearrange(
                        "p (r j) -> p r j", j=W
                    )
                    nc.vector.scalar_tensor_tensor(
                        out=ocv[:, :, 1:2], in0=utc[:, :, 0:1], scalar=-0.25,
                        in1=ocv[:, :, 1:2], op0=mult, op1=add,
                    )
                    nc.vector.scalar_tensor_tensor(
                        out=ocv[:, :, W - 2 : W - 1],
                        in0=utc[:, :, W - 1 : W], scalar=-0.25,
                        in1=ocv[:, :, W - 2 : W - 1], op0=mult, op1=add,
                    )
                    # zero output boundary
                    nc.vector.memset(ocv[:, :, 0:1], 0.0)
                    nc.vector.memset(ocv[:, :, W - 1 : W], 0.0)
                    if r0 == 0:
                        nc.vector.memset(oc[0:1, 0:W], 0.0)
                    if r0 + nr == RPP:
                        o_eng.dma_start(out=oc[127:128, cw - W : cw],
                                        in_=state["zto"])

                    o_eng.dma_start(out=o_img[:, r0 * W : r0 * W + cw], in_=oc)

            ut0 = issue_uload(0)
            issue_setup()
            issue_vrows(ut0)
            issue_chunks(0, ut0, final=(n_imgs == 1))
            for ib in range(1, n_imgs):
                ut = issue_uload(ib)
                issue_vrows(ut)
                issue_chunks(ib, ut, final=(ib == n_imgs - 1))
    nc.finalize()
    return nc


def _make_ident():
    ident = np.zeros((P, 3 * P), dtype=np.float32)
    ident[:, 0:P] = np.eye(P, dtype=np.float32)
    for p in range(1, P):
        ident[p - 1, P + p] = 1.0
    for p in range(P - 1):
        ident[p + 1, 2 * P + p] = 1.0
    return ident


def _get_nc(repeat=1, **kw):
    key = (repeat, tuple(sorted(kw.items())))
    if key not in _cache:
        _cache[key] = _build(repeat, **kw)
    return _cache[key]


def _in_maps(u, f):
    ident = _make_ident()
    return [
        {"u": u[i * IMGS : (i + 1) * IMGS], "f": f[i * IMGS : (i + 1) * IMGS],
         "ident": ident}
        for i in range(N_CORES)
    ]


def _run(u, f, trace=False, **kw):
    u = np.ascontiguousarray(np.asarray(u, dtype=np.float32))
    f = np.ascontiguousarray(np.asarray(f, dtype=np.float32))
    nc = _get_nc(**kw)
    res = run_bass_kernel_spmd(nc, _in_maps(u, f), core_ids=list(range(N_CORES)),
                               trace=trace)
    out = np.concatenate([r["out"] for r in res.results], axis=0)
    return out, res


def kernel(u, f, weight=None):
    out, _ = _run(u, f)
    return out
